# revision 33
# baseline (speedup 1.0000x reference)
"""Trainium2 Bass kernel for nn_ABCLayer (ABC-Net style binary conv layer).

Strategy: the layer is bilinear in the binarized weights/inputs, so
  y = sum_n beta_n sum_m alpha_m conv(bx_n, B_m) = conv(xb_eff, w_eff)
with w_eff = sum_m alpha_m sign(w - t_m) (a 6-level staircase of w) and
xb_eff = sum_n beta_n sign(clip(x + s_n, 0, 1) - 0.5) (4-level staircase).

All staircase/OLS parameter prep is tiny elementwise/scalar work and is
done on the host in make_in_maps (alphas via the exact 500-step OLS
recurrence using the 5x5 Gram matrix).  The device kernel is a pure
SAME-padded 3x3 conv at the tensor-engine roofline:
  - per-core f16 inputs: w_eff as (128, 9, 256) [ci, tap, co] and the
    padded image (128, 34, 36) [ci, r, c];
  - 4-queue DMA-in, two warm matmuls to ramp the PE p-state while the
    DMA lands, then 36 accumulating 128x128x512 matmuls (9 taps x
    2 Cout halves x 2 row halves); LDWEIGHTS hides under the previous
    matmul so the PE stays at ~1 row/cycle;
  - PSUM->SBUF f16 cast copies per output half overlap the remaining
    matmuls; 4 output DMAs on 2 queues.

Data parallel: core i processes image i; w_eff replicated.

Per-core layouts (host pre-transposes, gather post-transposes):
  weff : (128, 9, 256)  f16   weff[ci, tap, co]
  xb   : (128, 34, 36)  f16   padded image, pixel (r, c) at [ci, r+1, c+2]
  out  : (256, 1024)    f16   out[co, r*32+c] = y[i, r, c, co]
"""

import sys

if "/opt/trn_rl_repo" not in sys.path:
    sys.path.insert(0, "/opt/trn_rl_repo")

import numpy as np

import concourse.bass as bass  # noqa: E402
import concourse.tile as tile  # noqa: E402
from concourse import bacc, bass_utils, mybir  # noqa: E402

F32 = mybir.dt.float32
F16 = mybir.dt.float16

N_CORES = 8
B, H, W, CIN, COUT = 8, 32, 32, 128, 256
M, N = 5, 3
TAPS = 9
K = 3 * 3 * CIN * COUT  # 294912
LR = 0.01
NUM_EPOC = 500
PW = 36  # padded row length (2 zero cols left, 2 right)

_CACHE = {}


def build_nc():
    nc = bacc.Bacc("TRN2", target_bir_lowering=False, debug=False, num_devices=N_CORES)

    weff_d = nc.dram_tensor("weff", (CIN, TAPS, COUT), F16, kind="ExternalInput").ap()
    xb_d = nc.dram_tensor("xb", (CIN, H + 2, PW), F16, kind="ExternalInput").ap()
    out_d = nc.dram_tensor("out", (COUT, H * W), F16, kind="ExternalOutput").ap()

    with tile.TileContext(nc) as tc:
        with (
            tc.tile_pool(name="big", bufs=1) as big,
            tc.tile_pool(name="sm", bufs=1) as sm,
            tc.tile_pool(name="wps", bufs=1, space="PSUM") as wps,
            tc.tile_pool(name="cps", bufs=1, space="PSUM") as cps,
        ):
            weff = big.tile([CIN, TAPS, COUT], F16, tag="weff")
            xb = big.tile([CIN, H + 2, PW], F16, tag="xb")
            out_sb = big.tile([128, 2048], F16, tag="out_sb")
            warm_sb = sm.tile([128, 256], F16, tag="warm_sb")
            act_sb = sm.tile([1, 1], F16, tag="act_sb")
            warm_ps = wps.tile([128, 512], F32, tag="warm")
            pc = [[cps.tile([128, 512], F32, tag=f"pc{c}_{r}", name=f"pc{c}_{r}")
                   for r in range(2)] for c in range(2)]


            # nonzero warm data: all-zero operands produce no switching
            # activity in the PE array, giving the DVFS governor no power
            # signal to ramp the clock during the warm-up bridge
            nc.vector.memset(warm_sb[:, :], 1.0)

            # input DMAs on the two HW-DGE queues.  A transfer's completion
            # semaphores only flush after all data queued on its queue, so
            # the two conv-start-critical transfers go first and ALONE on
            # their queues; the rest are deferred until those complete.
            d_w03 = nc.sync.dma_start(out=weff[:, 0:3, :], in_=weff_d[:, 0:3, :])
            d_xbt = nc.scalar.dma_start(out=xb[:, 0:18, :], in_=xb_d[:, 0:18, :])
            d_w36 = nc.sync.dma_start(out=weff[:, 3:6, :], in_=weff_d[:, 3:6, :])
            tile.add_dep_helper(d_w36.ins, d_w03.ins, sync=True,
                                reason="keep weff03 sem flush clean")
            nc.sync.dma_start(out=weff[:, 6:TAPS, :], in_=weff_d[:, 6:TAPS, :])
            d_xbb = nc.scalar.dma_start(out=xb[:, 18:H + 2, :],
                                        in_=xb_d[:, 18:H + 2, :])
            tile.add_dep_helper(d_xbb.ins, d_xbt.ins, sync=True,
                                reason="keep xb-top sem flush clean")

            # preload the ACT function table off the critical path
            nc.scalar.copy(act_sb[:, :], warm_sb[0:1, 0:1])

            # warm matmuls: PE utilization speeds up the DVFS ramp and a
            # busy bridge avoids the cold-pipeline restart at conv start
            for _ in range(12):
                nc.tensor.matmul(warm_ps[:, 0:256], warm_sb[:, 0:128],
                                 warm_sb[:, 0:256])

            # conv: 4 passes of 9 taps, one PSUM quadrant per pass, so
            # output drains spread across the whole conv.  Pass order
            # (ch0,rh0) first: it only needs xb rows 0..17 + weff tap 0.
            # sequential passes: one PSUM quadrant per 9-tap pass, so the
            # first quadrant drains (copy + out-DMA) ~2.5us into the conv
            # and output traffic spreads across the whole conv
            prev_mm = [None]

            def mm(ch, rh, tap, start=False, stop=False):
                dy, dx = tap // 3, tap % 3
                r0 = rh * 16
                inst = nc.tensor.matmul(
                    pc[ch][rh][:, :],
                    weff[:, tap, ch * 128:(ch + 1) * 128],
                    xb[:, dy + r0:dy + r0 + 16, dx + 1:dx + 1 + W],
                    start=start, stop=stop)
                # pin the emission order: the tile scheduler otherwise
                # reorders independent matmuls and can hoist a quadrant
                # that waits on a late transfer ahead of ready ones
                if prev_mm[0] is not None:
                    tile.add_dep_helper(inst.ins, prev_mm[0].ins, sync=False,
                                        reason="keep conv matmul order")
                prev_mm[0] = inst

            def drain(qi, ch, rh):
                dst = out_sb[:, qi * 512:(qi + 1) * 512]
                od = out_d[ch * 128:(ch + 1) * 128, rh * 512:(rh + 1) * 512]
                if qi < 3:
                    if qi % 2 == 0:
                        nc.scalar.copy(dst, pc[ch][rh][:, :])
                    else:
                        nc.vector.tensor_copy(dst, pc[ch][rh][:, :])
                    (nc.sync if qi % 2 == 0 else nc.scalar).dma_start(
                        out=od, in_=dst)
                else:
                    # final quadrant: halves in parallel on both copy
                    # engines and both DMA queues to shorten the tail
                    nc.vector.tensor_copy(dst[:, 0:256], pc[ch][rh][:, 0:256])
                    nc.scalar.copy(dst[:, 256:512], pc[ch][rh][:, 256:512])
                    nc.sync.dma_start(out=od[:, 0:256], in_=dst[:, 0:256])
                    nc.scalar.dma_start(out=od[:, 256:512], in_=dst[:, 256:512])

            # the two xb-top quadrants interleave 3-tap blocks: at full
            # clock weff[3:6] is consumed at matmul 7 (~+1.4us), the
            # deferred weff[6:9] at matmul 13 (~+2.8us) and the xb bottom
            # half at matmul 19 (~+4us) — all after their ~+2.3us arrivals
            for tap in range(3):
                mm(0, 0, tap, start=(tap == 0))
            for tap in range(3):
                mm(1, 0, tap, start=(tap == 0))
            for tap in range(3, 6):
                mm(0, 0, tap)
            for tap in range(3, 6):
                mm(1, 0, tap)
            for tap in range(6, TAPS):
                mm(0, 0, tap, stop=(tap == TAPS - 1))
            drain(0, 0, 0)
            for tap in range(6, TAPS):
                mm(1, 0, tap, stop=(tap == TAPS - 1))
            drain(1, 1, 0)
            for tap in range(TAPS):
                mm(0, 1, tap, start=(tap == 0), stop=(tap == TAPS - 1))
            drain(2, 0, 1)
            for tap in range(TAPS):
                mm(1, 1, tap, start=(tap == 0), stop=(tap == TAPS - 1))
            drain(3, 1, 1)

    nc.compile()
    return nc


def make_in_maps(x, weight, shiftPara, beta, alphas_init):
    x = np.asarray(x, np.float32)
    w = np.asarray(weight, np.float32)
    shift = np.asarray(shiftPara, np.float32)
    beta_v = np.asarray(beta, np.float32)
    a0 = np.asarray(alphas_init, np.float64)

    # thresholds: sign(w - mean + s_m * sigma) = sign(w - (mean - s_m * sigma))
    mean = w.mean(dtype=np.float64)
    sig = np.sqrt(w.astype(np.float64).var())
    s = -1.0 + np.arange(M, dtype=np.float64) * (2.0 / (M - 1))
    thr = mean - s * sig  # (M,)

    # alphas: 500-step OLS GD in the 5-dim subspace (exact same recurrence)
    fw = w.reshape(-1).astype(np.float64)
    fb = np.sign(fw[None, :] - thr[:, None])  # (M, K)
    G = fb @ fb.T
    h = fb @ fw
    a = a0.copy()
    for _ in range(NUM_EPOC):
        a -= LR * (G @ a - h) / K

    # effective weights, transposed to [ci, tap, co]
    weff_flat = fb.T @ a  # (K,)
    weffT = np.ascontiguousarray(
        weff_flat.reshape(TAPS, CIN, COUT).transpose(1, 0, 2)).astype(np.float16)

    # effective binarized input
    xbe = np.zeros_like(x)
    for n in range(N):
        xbe += beta_v[n] * np.sign(
            np.clip(x + shift[n], 0.0, 1.0) - np.float32(0.5))

    in_maps = []
    for i in range(N_CORES):
        pad = np.zeros((CIN, H + 2, PW), np.float16)
        pad[:, 1:H + 1, 2:W + 2] = xbe[i].transpose(2, 0, 1)
        in_maps.append({"weff": weffT, "xb": pad})
    return in_maps


def kernel(x, weight, shiftPara, beta, alphas_init):
    if "nc" not in _CACHE:
        _CACHE["nc"] = build_nc()
    nc = _CACHE["nc"]
    in_maps = make_in_maps(x, weight, shiftPara, beta, alphas_init)
    res = bass_utils.run_bass_kernel_spmd(
        nc, in_maps, core_ids=list(range(N_CORES)))
    outs = [res.results[i]["out"] for i in range(N_CORES)]
    out = np.stack(outs, axis=0)  # (8, 256, 1024) f16
    out = out.transpose(0, 2, 1).reshape(B, H, W, COUT)
    return np.ascontiguousarray(out).astype(np.float32)
